# revision 65
# baseline (speedup 1.0000x reference)
"""DeepSAT GNN message-passing kernel for 8 Trainium2 NeuronCores.

Algorithm notes (validated numerically against the reference):
  - Every node is updated exactly once, at step l = forward_level (levels
    1..19; level-0 nodes keep h0 forever). At update time the node's own
    hidden state is still h0, so the GRU "hidden side" gates are constant
    vectors computable on the host.
  - msg_i = W @ (S_i + n0_i*h0) + deg_i*b, where S_i sums h over "active"
    in-edges (source level in [1, level_i)), n0_i counts inactive in-edges
    and deg_i all in-edges. With u = W^-1 b this folds to
    msg_i = W @ S'_i,  S'_i = S_i + n0_i*h0 + deg_i*u, so the per-gate
    input is  gi_g = (wih_g @ W) @ S'_i + bih_g  -- one fused matmul.
  - Nodes are stored level-sorted ("rank" order): per-level writes are
    contiguous, per-level ownership is an even 8-way split, and the
    AllGather of each level's new h lands in place.

Perf design (v2):
  - All matmuls/h-storage in bf16 (PE runs 1 cycle/row vs 4 for fp32);
    PSUM accumulation stays fp32.
  - Edge gathers use gpsimd.dma_gather (one SWDGE instruction per up to
    ~2-3k edges) instead of one indirect DMA per 128 edges: SWDGE fixed
    overhead is ~1us per instruction and was the previous bottleneck.
    dma_gather indices are int16, so "old" gathers (sources anywhere
    below the current level) are windowed into 32768-row slabs of
    h_store; "fresh" gathers (sources in level l-1 only) use the l-1
    slab directly.
  - Edges are sorted by destination slot so each 128-edge chunk spans
    few 128-slot blocks (~85% fill vs 44% before): fewer descriptors
    and fewer one-hot segment-sum matmuls.

Device schedule per level l (SPMD on 8 cores):
  dma_gather h[src] for this level's "fresh" edges (src level == l-1),
  segment-sum via one-hot matmuls into PSUM (seeded with the n0/deg
  terms), fused GRU, PE-transpose, DMA to the AllGather input, AllGather
  into the replicated h_store, then (overlapping the collective) the MLP
  head for this level plus the next level's "old" gathers/chunks, whose
  windows only read rows below this level's slab.
"""

import sys
import numpy as np

sys.path.insert(0, "/opt/trn_rl_repo")

P = 128
D = 128
NC = 8
GW = 512  # psum group width (one bank of fp32)
W32 = 32768  # dma_gather int16 index window

_COMPILED = {}


def _bf16():
    import ml_dtypes
    return ml_dtypes.bfloat16


# ---------------------------------------------------------------------------
# Host-side preprocessing
# ---------------------------------------------------------------------------

def _wrap_idx(vals):
    """int16 idx layout: j at [j%16, j//16], replicated to 8x16 partitions."""
    n = len(vals)
    cols = (n + 15) // 16
    t = np.zeros((P, cols), np.int16)
    for k in range(8):
        t[16 * k + (np.arange(n) % 16), np.arange(n) // 16] = vals
    return t


def _preprocess(forward_level, edge_index, num_levels):
    fl = np.asarray(forward_level).astype(np.int64)
    ei = np.asarray(edge_index).astype(np.int64)
    src, dst = ei[0], ei[1]
    N = fl.shape[0]
    NL = num_levels

    # --- rank space: nodes sorted by level, each level padded to NC*P ---
    n_l = np.bincount(fl, minlength=NL).astype(np.int64)
    pad_l = ((n_l + NC * P - 1) // (NC * P)) * (NC * P)
    pad_l = np.maximum(pad_l, NC * P)
    L_off = np.zeros(NL + 1, np.int64)
    L_off[1:] = np.cumsum(pad_l)
    Vc = (pad_l // NC).astype(np.int64)          # per-core nodes per level
    Voff = np.zeros(NL + 1, np.int64)
    Voff[1:] = np.cumsum(Vc)                     # per-core rank-space offsets
    nblk = (Vc // P).astype(np.int64)

    order = np.argsort(fl, kind="stable")
    starts_real = np.zeros(NL + 1, np.int64)
    starts_real[1:] = np.cumsum(n_l)
    pos_within = np.arange(N, dtype=np.int64) - starts_real[fl[order]]
    rank = np.empty(N, np.int64)
    rank[order] = L_off[fl[order]] + pos_within

    node_of_rank = np.full(L_off[NL], -1, np.int64)
    node_of_rank[rank] = np.arange(N, dtype=np.int64)

    # --- per-node degree stats, indexed by per-core rank space ---
    lv_s, lv_d = fl[src], fl[dst]
    act = (lv_s >= 1) & (lv_s < lv_d)
    deg = np.bincount(dst, minlength=N).astype(np.float64)
    n0 = np.bincount(dst[~act], minlength=N).astype(np.float64)

    sumVc = int(Voff[NL])
    n0row = np.zeros((NC, sumVc), np.float32)
    degrow = np.zeros((NC, sumVc), np.float32)
    for c in range(NC):
        grs = []
        for l in range(NL):
            grs.append(L_off[l] + c * Vc[l] + np.arange(Vc[l]))
        gr = np.concatenate(grs)
        nd = node_of_rank[gr]
        m = nd >= 0
        n0row[c, m] = n0[nd[m]]
        degrow[c, m] = deg[nd[m]]

    # --- active edge table ---
    er = np.where(act)[0]
    e_lvl = lv_d[er]
    e_srcrank = rank[src[er]].astype(np.int64)
    e_dstrank = rank[dst[er]].astype(np.int64)
    e_local = e_dstrank - L_off[e_lvl]
    e_core = e_local // Vc[e_lvl]
    e_wl = e_local % Vc[e_lvl]          # slot within core's level span
    e_fresh = lv_s[er] == (e_lvl - 1)

    # Per (level, phase[, window]) gather instructions; edges sorted by dst
    # slot so chunks span few blocks. All counts are the max over cores so
    # the SPMD program is identical everywhere.
    idx_cols = []                # list of [128, n/16] int16 blocks
    icol = 0
    rank_cols = [[] for _ in range(NC)]  # per pair: [128] f32 block-rel slots
    levels = []
    for l in range(NL):
        info = {"instrs": [], "pairs": [], "last": {},
                "ngrp": (int(Vc[l]) + GW - 1) // GW,
                "fresh_chunk0": 0, "old_chunk0": 0,
                "nfresh_chunks": 0, "nold_chunks": 0}
        if l >= 1:
            in_lvl = e_lvl == l
            # bucket list: ("fresh", base, rows, sel) or ("old", w, ...)
            buckets = []
            fsel = in_lvl & e_fresh
            if l >= 2:
                base = int(L_off[l - 1])
                assert pad_l[l - 1] <= W32, "fresh slab exceeds int16 idx range"
                buckets.append(("fresh", base, int(pad_l[l - 1]), fsel, 0))
            osel = in_lvl & ~e_fresh
            if l >= 3:
                max_row = int(L_off[l - 1])   # old srcs are below lvl l-1
                nw = (max_row + W32 - 1) // W32
                for w in range(nw):
                    wsel = osel & (e_srcrank >= w * W32) & (e_srcrank < (w + 1) * W32)
                    rows = min(W32, int(L_off[NL]) - w * W32)
                    # pad idx must point at an already-written row: window 0
                    # starts in the never-written level-0 slab.
                    padidx = int(L_off[1]) if w == 0 else 0
                    buckets.append(("old", w * W32, rows, wsel, padidx))
            fresh_chunks = 0
            old_chunks = 0
            for phase, base, rows, sel, padidx in buckets:
                percore = []
                for c in range(NC):
                    es = np.where(sel & (e_core == c))[0]
                    es = es[np.argsort(e_wl[es], kind="stable")]
                    percore.append(es)
                nmax = max(len(x) for x in percore)
                if nmax == 0:
                    continue
                n = ((nmax + P - 1) // P) * P
                nch = n // P
                # idx block (same wrapped layout for every core? no - idxs
                # differ per core; build per core below)
                iv = np.zeros((NC, n), np.int64)
                for c in range(NC):
                    es = percore[c]
                    iv[c, :len(es)] = e_srcrank[es] - base
                    iv[c, len(es):] = padidx
                assert iv.min() >= 0 and iv.max() < min(rows, W32)
                ch0 = fresh_chunks if phase == "fresh" else old_chunks
                info["instrs"].append({
                    "phase": phase, "base": base, "rows": rows, "n": n,
                    "icol": icol, "chunk0": ch0,
                })
                idx_cols.append(iv)
                icol += n // 16
                # pairs: for each chunk, union over cores of touched blocks
                for ch in range(nch):
                    sl = slice(ch * P, (ch + 1) * P)
                    blocks = set()
                    for c in range(NC):
                        es = percore[c][sl]
                        blocks.update(np.unique(e_wl[es] // P).tolist())
                    for b in sorted(blocks):
                        pcol = len(rank_cols[0])
                        for c in range(NC):
                            es = percore[c][sl]
                            rv = np.full(P, -1.0, np.float32)
                            wl = e_wl[es]
                            m = (wl // P) == b
                            rv[:len(es)][m] = (wl[m] - b * P).astype(np.float32)
                            rank_cols[c].append(rv)
                        info["pairs"].append(
                            (phase, ch0 + ch, int(b) // 4, int(b) % 4, pcol))
                if phase == "fresh":
                    fresh_chunks += nch
                else:
                    old_chunks += nch
            info["nfresh_chunks"] = fresh_chunks
            info["nold_chunks"] = old_chunks
            # last matmul per psum group in EMISSION order: old pairs are
            # emitted (at level l-1) before fresh pairs (at level l)
            for want in ("old", "fresh"):
                for phase, ch, grp, big, pcol in info["pairs"]:
                    if phase == want:
                        info["last"][grp] = (phase, pcol)
        levels.append(info)

    ICOLS = max(icol, 1)
    idxs = np.zeros((NC, P, ICOLS), np.int16)
    col = 0
    bi = 0
    for l in range(NL):
        for ins in levels[l]["instrs"]:
            iv = idx_cols[bi]
            bi += 1
            n = iv.shape[1]
            cols = n // 16
            for c in range(NC):
                idxs[c][:, ins["icol"]:ins["icol"] + cols] = _wrap_idx(iv[c])
            col += cols
    NPAIR = max(len(rank_cols[0]), 1)
    ranks = np.full((NC, P, NPAIR), -1.0, np.float32)
    for c in range(NC):
        if rank_cols[c]:
            ranks[c, :, :len(rank_cols[c])] = np.stack(rank_cols[c], axis=1)

    return {
        "N": N, "NL": NL, "n_l": n_l, "pad": pad_l, "L_off": L_off,
        "Vc": Vc, "Voff": Voff, "nblk": nblk, "sumVc": sumVc,
        "ICOLS": ICOLS, "NPAIR": NPAIR,
        "levels": levels, "idxs": idxs, "ranks": ranks,
        "n0row": n0row, "degrow": degrow, "node_of_rank": node_of_rank,
    }


def _prep_weights(inp):
    f64 = np.float64
    W = inp["aggr_w"].astype(f64)
    b = inp["aggr_b"].astype(f64)
    h0 = (inp["emd_w"][:, 0] + inp["emd_b"]).astype(f64)
    wih = inp["gru_wih"].astype(f64)
    whh = inp["gru_whh"].astype(f64)
    bih = inp["gru_bih"].astype(f64)
    bhh = inp["gru_bhh"].astype(f64)
    u = np.linalg.solve(W, b)
    assert np.abs(W @ u - b).max() < 1e-6
    ghc = whh @ h0 + bhh
    hr_c, hz_c, hn_c = ghc[:D], ghc[D:2 * D], ghc[2 * D:]
    bih_r, bih_z, bih_n = bih[:D], bih[D:2 * D], bih[2 * D:]
    WgT = [(wih[g * D:(g + 1) * D] @ W).T for g in range(3)]

    W1 = inp["w1"].astype(f64)  # [256, 128]
    W2 = inp["w2"].astype(f64)  # [256, 256]
    w3 = inp["w3"].astype(f64)  # [1, 256]
    assert W1.shape[0] == 256

    bf16 = _bf16()
    blocks = [
        WgT[0], WgT[1], WgT[2], np.diag(hn_c),
        W1[0:128, :].T, W1[128:256, :].T,
        W2[0:128, 0:128].T, W2[0:128, 128:256].T,
        W2[128:256, 0:128].T, W2[128:256, 128:256].T,
        np.eye(128), np.tile(np.arange(128, dtype=f64)[None, :], (128, 1)),
        np.concatenate([w3[0, 0:128, None], w3[0, 128:256, None],
                        np.zeros((128, 126))], axis=1),
    ]
    wmat = np.concatenate(blocks, axis=1).astype(bf16)  # [128, 13*128] bf16

    vcols = np.stack([
        h0,                      # 0: h0 column
        bih_r + hr_c,            # 1: sigmoid bias for r
        -(bih_z + hz_c),         # 2: sigmoid bias for z' (scale = -1)
        bih_n,                   # 3: tanh bias for n
        inp["b1"].astype(f64)[0:128],    # 4
        inp["b1"].astype(f64)[128:256],  # 5
        inp["b2"].astype(f64)[0:128],    # 6
        inp["b2"].astype(f64)[128:256],  # 7
        np.full(128, inp["b3"].astype(f64)[0]),  # 8: b3 (row 0 used)
    ], axis=1).astype(np.float32)  # [128, 9] fp32 (activation biases + h0)

    vrow = np.zeros((1, 256), np.float32)
    vrow[0, :128] = h0.astype(np.float32)
    vrow[0, 128:] = u.astype(np.float32)
    return wmat, vcols, vrow.astype(bf16)


# ---------------------------------------------------------------------------
# Bass program
# ---------------------------------------------------------------------------

WM = {name: i for i, name in enumerate(
    ["WgT_r", "WgT_z", "WgT_n", "diag_hn", "W1Ta", "W1Tb",
     "W2_k0m0", "W2_k1m0", "W2_k0m1", "W2_k1m1", "ident", "iota", "w3c"])}
VC = {name: i for i, name in enumerate(
    ["h0", "bias_r", "nbias_z", "bias_n", "b1a", "b1b", "b2a", "b2b", "b3"])}


def _build(sched, reps=1):
    import concourse.bacc as bacc
    import concourse.tile as tile
    from concourse import bass, mybir, library_config

    f32 = mybir.dt.float32
    bf = mybir.dt.bfloat16
    i16 = mybir.dt.int16
    AF = mybir.ActivationFunctionType
    OP = mybir.AluOpType
    NL = sched["NL"]
    L_off = sched["L_off"]
    Vc = sched["Vc"]
    Voff = sched["Voff"]
    pad = sched["pad"]
    ICOLS = sched["ICOLS"]
    NPAIR = sched["NPAIR"]
    sumVc = sched["sumVc"]
    NpadTot = int(L_off[NL])
    RG = [list(range(NC))]

    nc = bacc.Bacc("TRN2", target_bir_lowering=False, debug=False,
                   enable_asserts=False, num_devices=NC)

    wmat_d = nc.dram_tensor("wmat", [P, P * len(WM)], bf, kind="ExternalInput")
    vcols_d = nc.dram_tensor("vcols", [P, len(VC)], f32, kind="ExternalInput")
    vrow_d = nc.dram_tensor("vrow", [1, 256], bf, kind="ExternalInput")
    n0_d = nc.dram_tensor("n0row", [1, sumVc], bf, kind="ExternalInput")
    deg_d = nc.dram_tensor("degrow", [1, sumVc], bf, kind="ExternalInput")
    idx_d = nc.dram_tensor("idxs", [P, ICOLS], i16, kind="ExternalInput")
    rnk_d = nc.dram_tensor("ranks", [P, NPAIR], bf, kind="ExternalInput")
    pred_d = nc.dram_tensor("pred", [sumVc], f32, kind="ExternalOutput")
    h_store = nc.dram_tensor("h_store", [NpadTot, D], bf, kind="Internal",
                             addr_space="Shared")
    ag_in = [nc.dram_tensor(f"ag_in{i}", [int(Vc.max()), D], bf, kind="Internal")
             for i in range(2)]

    with tile.TileContext(nc) as tc:
        nc.gpsimd.load_library(library_config.mlp)
        cpool = tc.alloc_tile_pool(name="const", bufs=1)
        spool = tc.alloc_tile_pool(name="sbuf", bufs=2)
        gpool = tc.alloc_tile_pool(name="gath", bufs=2)
        hpool = tc.alloc_tile_pool(name="hnew", bufs=6)
        ppool = tc.alloc_tile_pool(name="psS", bufs=3, space="PSUM")
        qpool = tc.alloc_tile_pool(name="psG", bufs=3, space="PSUM")
        tpool = tc.alloc_tile_pool(name="psT", bufs=1, space="PSUM")
        rpool = tc.alloc_tile_pool(name="psP", bufs=1, space="PSUM")

        # ---- load constants ----
        wm = cpool.tile([P, P * len(WM)], bf, tag="wm")
        nc.sync.dma_start(out=wm[:], in_=wmat_d[:])
        vc = cpool.tile([P, len(VC)], f32, tag="vc")
        nc.sync.dma_start(out=vc[:], in_=vcols_d[:])
        vr = cpool.tile([1, 256], bf, tag="vr")
        nc.sync.dma_start(out=vr[:], in_=vrow_d[:])
        idxs = cpool.tile([P, ICOLS], i16, tag="idxs")
        nc.sync.dma_start(out=idxs[:], in_=idx_d[:])
        rnks = cpool.tile([P, NPAIR], bf, tag="rnks")
        nc.sync.dma_start(out=rnks[:], in_=rnk_d[:])

        def wmb(name):
            return wm[:, WM[name] * P:(WM[name] + 1) * P]

        def vcc(name):
            return vc[:, VC[name]:VC[name] + 1]

        h0b = cpool.tile([P, GW], bf, tag="h0b")  # h0 broadcast along free
        nc.vector.tensor_copy(out=h0b[:], in_=vcc("h0").to_broadcast([P, GW]))

        # ---- per-level state ----
        S_ps = [None] * NL         # list of psum tiles per level (by grp)
        last_ag = [None]           # most recent AllGather instruction

        def grp_widths(l):
            ws = []
            v = int(Vc[l])
            while v > 0:
                ws.append(min(GW, v))
                v -= GW
            return ws

        def emit_gathers(l, phase):
            """dma_gather(s) for one phase of level l."""
            if l >= NL:
                return None
            info = sched["levels"][l]
            nch = info["nfresh_chunks" if phase == "fresh" else "nold_chunks"]
            if nch == 0:
                return None
            hg = gpool.tile([P, nch * D], bf, tag="hg_" + phase)
            for ins in info["instrs"]:
                if ins["phase"] != phase:
                    continue
                n = ins["n"]
                c0 = ins["chunk0"]
                gi = nc.gpsimd.dma_gather(
                    out_ap=hg[:, c0 * D:(c0 + n // P) * D].rearrange(
                        "p (k d) -> p k d", d=D),
                    in_ap=h_store[ins["base"]:ins["base"] + ins["rows"], :],
                    idxs_ap=idxs[:, ins["icol"]:ins["icol"] + n // 16],
                    num_idxs=n,
                    num_idxs_reg=n,
                    elem_size=D,
                )
                # the windowed read of h_store races the AllGathers unless
                # pinned by hand (DRAM regions aren't shadow-tracked)
                if last_ag[0] is not None:
                    tile.add_dep_helper(gi.ins, last_ag[0].ins, sync=True,
                                        reason="gather reads AllGather output")
            return hg

        def emit_onehots(l, phase):
            info = sched["levels"][l]
            pairs = [p for p in info["pairs"] if p[0] == phase]
            if not pairs:
                return None, None
            k = len(pairs)
            p0 = pairs[0][4]
            oh = spool.tile([P, k * P], bf, tag="oh_" + phase)
            CH = 4
            for s in range(0, k, CH):
                m = min(CH, k - s)
                nc.vector.tensor_tensor(
                    out=oh[:, s * P:(s + m) * P].rearrange("p (m f) -> p m f", m=m),
                    in0=rnks[:, p0 + s:p0 + s + m][:, :, None].to_broadcast([P, m, P]),
                    in1=wmb("iota")[:, None, :].to_broadcast([P, m, P]),
                    op=OP.is_equal,
                )
            return oh, p0

        def emit_seeds(l):
            """allocate S psums for level l and seed with n0*h0 + deg*u."""
            tiles = []
            info = sched["levels"][l]
            v = int(Vc[l])
            off = int(Voff[l])
            n0r = spool.tile([1, int(Vc.max())], bf, tag="n0r")
            nc.sync.dma_start(out=n0r[0:1, :v], in_=n0_d[0:1, off:off + v])
            degr = spool.tile([1, int(Vc.max())], bf, tag="degr")
            nc.sync.dma_start(out=degr[0:1, :v], in_=deg_d[0:1, off:off + v])
            for g, w in enumerate(grp_widths(l)):
                sp = ppool.tile([P, GW], f32, tag="S", space="PSUM")
                nc.tensor.matmul(
                    out=sp[:, :w], lhsT=vr[0:1, 0:128],
                    rhs=n0r[0:1, g * GW:g * GW + w],
                    start=True, stop=False, skip_group_check=True)
                is_last = info["last"].get(g) is None
                nc.tensor.matmul(
                    out=sp[:, :w], lhsT=vr[0:1, 128:256],
                    rhs=degr[0:1, g * GW:g * GW + w],
                    start=False, stop=is_last, skip_group_check=True)
                tiles.append(sp)
            S_ps[l] = tiles

        def emit_chunks(l, phase, hg, oh, p0):
            info = sched["levels"][l]
            pairs = [p for p in info["pairs"] if p[0] == phase]
            if not pairs:
                return
            for (ph, ch, grp, big, pcol) in pairs:
                is_last = info["last"].get(grp) == (phase, pcol)
                nc.tensor.matmul(
                    out=S_ps[l][grp][:, big * P:(big + 1) * P],
                    lhsT=hg[:, ch * D:(ch + 1) * D],
                    rhs=oh[:, (pcol - p0) * P:(pcol - p0 + 1) * P],
                    start=False, stop=is_last, skip_group_check=True)

        def emit_mlp(l, g, w, rhs_sb, bcast=False):
            """MLP head for one 512-group; writes pred rows."""
            z1s = []
            for half in ("a", "b"):
                zp = qpool.tile([P, GW], f32, tag="G", space="PSUM")
                nc.tensor.matmul(out=zp[:, :w], lhsT=wmb("W1T" + half),
                                 rhs=rhs_sb[:, :w], start=True, stop=True)
                zs = spool.tile([P, GW], bf, tag="z1" + half)
                nc.scalar.activation(out=zs[:, :w], in_=zp[:, :w], func=AF.Relu,
                                     bias=vcc("b1" + half))
                z1s.append(zs)
            z2s = []
            for mi, mh in enumerate(("m0", "m1")):
                zp = qpool.tile([P, GW], f32, tag="G", space="PSUM")
                nc.tensor.matmul(out=zp[:, :w], lhsT=wmb("W2_k0" + mh),
                                 rhs=z1s[0][:, :w], start=True, stop=False)
                nc.tensor.matmul(out=zp[:, :w], lhsT=wmb("W2_k1" + mh),
                                 rhs=z1s[1][:, :w], start=False, stop=True)
                zs = spool.tile([P, GW], bf, tag="z2" + mh)
                nc.scalar.activation(out=zs[:, :w], in_=zp[:, :w], func=AF.Relu,
                                     bias=vcc("b2" + ("a" if mi == 0 else "b")))
                z2s.append(zs)
            pp = rpool.tile([1, GW], f32, tag="pred", space="PSUM")
            nc.tensor.matmul(out=pp[:, :w], lhsT=wmb("w3c")[:, 0:1],
                             rhs=z2s[0][:, :w], start=True, stop=False)
            nc.tensor.matmul(out=pp[:, :w], lhsT=wmb("w3c")[:, 1:2],
                             rhs=z2s[1][:, :w], start=False, stop=True)
            ps = spool.tile([1, GW], f32, tag="psb")
            nc.scalar.activation(out=ps[:, :w], in_=pp[:, :w], func=AF.Identity,
                                 bias=vc[0:1, VC["b3"]:VC["b3"] + 1])
            if bcast:
                pbc = spool.tile([1, GW], f32, tag="pbc")
                nc.vector.tensor_copy(out=pbc[0:1, :],
                                      in_=ps[0:1, 0:1].to_broadcast([1, GW]))
                for gg, ww in enumerate(grp_widths(l)):
                    off = int(Voff[l]) + gg * GW
                    nc.sync.dma_start(out=pred_d[off:off + ww],
                                      in_=pbc[0:1, :ww])
            else:
                off = int(Voff[l]) + g * GW
                nc.sync.dma_start(out=pred_d[off:off + w], in_=ps[0:1, :w])

        # reps>1 repeats the whole computation for wall-clock timing: the
        # computation is idempotent (h_store/pred rewritten with same values)
        for _rep in range(reps):
          # ================= level 0: one column, broadcast ==============
          # every level-0 node keeps h = h0, so pred is a single scalar
          emit_mlp(0, 0, 1, h0b, bcast=True)

          # seeds + (no old/fresh chunks possible) for level 1
          emit_seeds(1)
          Old_sb = [None] * (NL + 1)
          OH = {}  # one-hots, generated one iteration ahead

          def prefetch_oh(t):
              if t < NL:
                  OH[(t, "f")] = emit_onehots(t, "fresh")
                  if t + 1 < NL:
                      OH[(t + 1, "o")] = emit_onehots(t + 1, "old")

          prefetch_oh(1)

          # ================= levels 1..NL-1 =================
          for l in range(1, NL):
            widths = grp_widths(l)

            # fresh gather + chunks for this level
            hg_f = emit_gathers(l, "fresh")
            oh_f, p0_f = OH.get((l, "f"), (None, None))
            if hg_f is not None:
                emit_chunks(l, "fresh", hg_f, oh_f, p0_f)

            # old gather for next level: its sources are at levels <= l-1,
            # i.e. rows below L_off[l], so it overlaps this level's AllGather
            if l + 1 < NL:
                Old_sb[l + 1] = emit_gathers(l + 1, "old")

            # GRU per group
            hnew = []
            for g, w in enumerate(widths):
                veng = nc.vector
                ssb = spool.tile([P, GW], bf, tag="Ssb")
                nc.vector.tensor_copy(out=ssb[:, :w], in_=S_ps[l][g][:, :w])

                gr = qpool.tile([P, GW], f32, tag="G", space="PSUM")
                nc.tensor.matmul(out=gr[:, :w], lhsT=wmb("WgT_r"),
                                 rhs=ssb[:, :w], start=True, stop=True)
                gz = qpool.tile([P, GW], f32, tag="G", space="PSUM")
                nc.tensor.matmul(out=gz[:, :w], lhsT=wmb("WgT_z"),
                                 rhs=ssb[:, :w], start=True, stop=True)
                gn = qpool.tile([P, GW], f32, tag="G", space="PSUM")
                nc.tensor.matmul(out=gn[:, :w], lhsT=wmb("WgT_n"),
                                 rhs=ssb[:, :w], start=True, stop=False)

                rsb = spool.tile([P, GW], bf, tag="rsb")
                nc.scalar.activation(out=rsb[:, :w], in_=gr[:, :w],
                                     func=AF.Sigmoid, bias=vcc("bias_r"))
                zsb = spool.tile([P, GW], bf, tag="zsb")
                nc.scalar.activation(out=zsb[:, :w], in_=gz[:, :w],
                                     func=AF.Sigmoid, bias=vcc("nbias_z"),
                                     scale=-1.0)
                nc.tensor.matmul(out=gn[:, :w], lhsT=wmb("diag_hn"),
                                 rhs=rsb[:, :w], start=False, stop=True)
                nsb = spool.tile([P, GW], bf, tag="nsb")
                nc.scalar.activation(out=nsb[:, :w], in_=gn[:, :w],
                                     func=AF.Tanh, bias=vcc("bias_n"))

                t3 = spool.tile([P, GW], bf, tag="t3")
                veng.tensor_scalar(out=t3[:, :w], in0=nsb[:, :w],
                                   scalar1=vcc("h0"), scalar2=None,
                                   op0=OP.subtract)
                t4 = spool.tile([P, GW], bf, tag="t4")
                veng.tensor_tensor(out=t4[:, :w], in0=t3[:, :w],
                                   in1=zsb[:, :w], op=OP.mult)
                hn = hpool.tile([P, GW], bf, tag="hnew")
                veng.tensor_scalar(out=hn[:, :w], in0=t4[:, :w],
                                   scalar1=vcc("h0"), scalar2=None,
                                   op0=OP.add)
                hnew.append(hn)

            # transpose h_new to node-major, stage straight from PSUM, and
            # AllGather into every core's h_store (skipped for the last
            # level: nothing reads it)
            if l < NL - 1:
                agt = ag_in[l % 2]
                for g, w in enumerate(widths):
                    tp = tpool.tile([P, GW], bf, tag="tp", space="PSUM")
                    nb = w // P
                    for b in range(nb):
                        nc.tensor.transpose(
                            out=tp[:, b * P:(b + 1) * P],
                            in_=hnew[g][:, b * P:(b + 1) * P],
                            identity=wmb("ident"))
                    tps = spool.tile([P, GW], bf, tag="tps")
                    nc.vector.tensor_copy(out=tps[:, :w], in_=tp[:, :w])
                    for b in range(nb):
                        row = g * GW + b * P
                        nc.sync.dma_start(out=agt[row:row + P, :],
                                          in_=tps[:, b * P:(b + 1) * P])
                cc = nc.gpsimd.collective_compute(
                    "AllGather", mybir.AluOpType.bypass,
                    replica_groups=RG,
                    ins=[agt[0:int(Vc[l]), :].opt()],
                    outs=[h_store[int(L_off[l]):int(L_off[l]) + int(pad[l]), :].opt()],
                )
                last_ag[0] = cc

            # MLP head for this level (fills the AllGather latency)
            for g, w in enumerate(widths):
                emit_mlp(l, g, w, hnew[g])

            # seeds + old chunks for the next level (also fill the AllGather)
            if l + 1 < NL:
                emit_seeds(l + 1)
                oh_o, p0_o = OH.get((l + 1, "o"), (None, None))
                if Old_sb[l + 1] is not None:
                    emit_chunks(l + 1, "old", Old_sb[l + 1], oh_o, p0_o)

            prefetch_oh(l + 1)

        for pl in (rpool, tpool, qpool, ppool, hpool, gpool, spool, cpool):
            pl.release()

    nc.compile()
    return nc


# ---------------------------------------------------------------------------
# Entry point
# ---------------------------------------------------------------------------

def _run(inputs, trace=False, reps=1):
    from concourse.bass_utils import run_bass_kernel_spmd

    inputs = {k: np.asarray(v) for k, v in inputs.items()}
    bf16 = _bf16()
    fl = np.asarray(inputs["forward_level"])
    num_levels = int(fl.max()) + 1
    sched = _preprocess(fl, inputs["edge_index"], num_levels)
    wmat, vcols, vrow = _prep_weights(inputs)

    key = (sched["N"], sched["ICOLS"], sched["NPAIR"], sched["sumVc"], reps,
           tuple(int(x) for x in sched["Vc"]),
           tuple((len(i["instrs"]), len(i["pairs"]))
                 for i in sched["levels"]))
    if key not in _COMPILED:
        _COMPILED[key] = _build(sched, reps=reps)
    nc = _COMPILED[key]

    in_maps = []
    for c in range(NC):
        in_maps.append({
            "wmat": wmat, "vcols": vcols, "vrow": vrow,
            "n0row": sched["n0row"][c][None, :].astype(bf16),
            "degrow": sched["degrow"][c][None, :].astype(bf16),
            "idxs": sched["idxs"][c],
            "ranks": sched["ranks"][c].astype(bf16),
        })

    res = run_bass_kernel_spmd(nc, in_maps, core_ids=list(range(NC)),
                               trace=trace)

    NL = sched["NL"]
    L_off, Vc, Voff = sched["L_off"], sched["Vc"], sched["Voff"]
    node_of_rank = sched["node_of_rank"]
    out = np.zeros(sched["N"], np.float32)
    for c in range(NC):
        oc = res.results[c]["pred"]
        for l in range(NL):
            gr = int(L_off[l]) + c * int(Vc[l]) + np.arange(int(Vc[l]))
            nd = node_of_rank[gr]
            m = nd >= 0
            out[nd[m]] = oc[int(Voff[l]):int(Voff[l]) + int(Vc[l])][m]
    return out[:, None], res


def kernel(**inputs):
    out, _ = _run(inputs, trace=False)
    return out


# revision 66
# speedup vs baseline: 1.0065x; 1.0065x over previous
"""DeepSAT GNN message-passing kernel for 8 Trainium2 NeuronCores.

Algorithm notes (validated numerically against the reference):
  - Every node is updated exactly once, at step l = forward_level (levels
    1..19; level-0 nodes keep h0 forever). At update time the node's own
    hidden state is still h0, so the GRU "hidden side" gates are constant
    vectors computable on the host.
  - msg_i = W @ (S_i + n0_i*h0) + deg_i*b, where S_i sums h over "active"
    in-edges (source level in [1, level_i)), n0_i counts inactive in-edges
    and deg_i all in-edges. With u = W^-1 b this folds to
    msg_i = W @ S'_i,  S'_i = S_i + n0_i*h0 + deg_i*u, so the per-gate
    input is  gi_g = (wih_g @ W) @ S'_i + bih_g  -- one fused matmul.
  - Nodes are stored level-sorted ("rank" order): per-level writes are
    contiguous, per-level ownership is an even 8-way split, and the
    AllGather of each level's new h lands in place.

Perf design (v2):
  - All matmuls/h-storage in bf16 (PE runs 1 cycle/row vs 4 for fp32);
    PSUM accumulation stays fp32.
  - Edge gathers use gpsimd.dma_gather (one SWDGE instruction per up to
    ~2-3k edges) instead of one indirect DMA per 128 edges: SWDGE fixed
    overhead is ~1us per instruction and was the previous bottleneck.
    dma_gather indices are int16, so "old" gathers (sources anywhere
    below the current level) are windowed into 32768-row slabs of
    h_store; "fresh" gathers (sources in level l-1 only) use the l-1
    slab directly.
  - Edges are sorted by destination slot so each 128-edge chunk spans
    few 128-slot blocks (~85% fill vs 44% before): fewer descriptors
    and fewer one-hot segment-sum matmuls.

Device schedule per level l (SPMD on 8 cores):
  dma_gather h[src] for this level's "fresh" edges (src level == l-1),
  segment-sum via one-hot matmuls into PSUM (seeded with the n0/deg
  terms), fused GRU, PE-transpose, DMA to the AllGather input, AllGather
  into the replicated h_store, then (overlapping the collective) the MLP
  head for this level plus the next level's "old" gathers/chunks, whose
  windows only read rows below this level's slab.
"""

import sys
import numpy as np

sys.path.insert(0, "/opt/trn_rl_repo")

P = 128
D = 128
NC = 8
GW = 512  # psum group width (one bank of fp32)
W32 = 32768  # dma_gather int16 index window

_COMPILED = {}


def _bf16():
    import ml_dtypes
    return ml_dtypes.bfloat16


# ---------------------------------------------------------------------------
# Host-side preprocessing
# ---------------------------------------------------------------------------

def _wrap_idx(vals):
    """int16 idx layout: j at [j%16, j//16], replicated to 8x16 partitions."""
    n = len(vals)
    cols = (n + 15) // 16
    t = np.zeros((P, cols), np.int16)
    for k in range(8):
        t[16 * k + (np.arange(n) % 16), np.arange(n) // 16] = vals
    return t


def _preprocess(forward_level, edge_index, num_levels):
    fl = np.asarray(forward_level).astype(np.int64)
    ei = np.asarray(edge_index).astype(np.int64)
    src, dst = ei[0], ei[1]
    N = fl.shape[0]
    NL = num_levels

    # --- rank space: nodes sorted by level, each level padded to NC*P ---
    n_l = np.bincount(fl, minlength=NL).astype(np.int64)
    pad_l = ((n_l + NC * P - 1) // (NC * P)) * (NC * P)
    pad_l = np.maximum(pad_l, NC * P)
    L_off = np.zeros(NL + 1, np.int64)
    L_off[1:] = np.cumsum(pad_l)
    Vc = (pad_l // NC).astype(np.int64)          # per-core nodes per level
    Voff = np.zeros(NL + 1, np.int64)
    Voff[1:] = np.cumsum(Vc)                     # per-core rank-space offsets
    nblk = (Vc // P).astype(np.int64)

    order = np.argsort(fl, kind="stable")
    starts_real = np.zeros(NL + 1, np.int64)
    starts_real[1:] = np.cumsum(n_l)
    pos_within = np.arange(N, dtype=np.int64) - starts_real[fl[order]]
    rank = np.empty(N, np.int64)
    rank[order] = L_off[fl[order]] + pos_within

    node_of_rank = np.full(L_off[NL], -1, np.int64)
    node_of_rank[rank] = np.arange(N, dtype=np.int64)

    # --- per-node degree stats, indexed by per-core rank space ---
    lv_s, lv_d = fl[src], fl[dst]
    act = (lv_s >= 1) & (lv_s < lv_d)
    deg = np.bincount(dst, minlength=N).astype(np.float64)
    n0 = np.bincount(dst[~act], minlength=N).astype(np.float64)

    sumVc = int(Voff[NL])
    n0row = np.zeros((NC, sumVc), np.float32)
    degrow = np.zeros((NC, sumVc), np.float32)
    for c in range(NC):
        grs = []
        for l in range(NL):
            grs.append(L_off[l] + c * Vc[l] + np.arange(Vc[l]))
        gr = np.concatenate(grs)
        nd = node_of_rank[gr]
        m = nd >= 0
        n0row[c, m] = n0[nd[m]]
        degrow[c, m] = deg[nd[m]]

    # --- active edge table ---
    er = np.where(act)[0]
    e_lvl = lv_d[er]
    e_srcrank = rank[src[er]].astype(np.int64)
    e_dstrank = rank[dst[er]].astype(np.int64)
    e_local = e_dstrank - L_off[e_lvl]
    e_core = e_local // Vc[e_lvl]
    e_wl = e_local % Vc[e_lvl]          # slot within core's level span
    e_fresh = lv_s[er] == (e_lvl - 1)

    # Per (level, phase[, window]) gather instructions; edges sorted by dst
    # slot so chunks span few blocks. All counts are the max over cores so
    # the SPMD program is identical everywhere.
    idx_cols = []                # list of [128, n/16] int16 blocks
    icol = 0
    rank_cols = [[] for _ in range(NC)]  # per pair: [128] f32 block-rel slots
    levels = []
    for l in range(NL):
        info = {"instrs": [], "pairs": [], "last": {},
                "ngrp": (int(Vc[l]) + GW - 1) // GW,
                "fresh_chunk0": 0, "old_chunk0": 0,
                "nfresh_chunks": 0, "nold_chunks": 0}
        if l >= 1:
            in_lvl = e_lvl == l
            # bucket list: ("fresh", base, rows, sel) or ("old", w, ...)
            buckets = []
            fsel = in_lvl & e_fresh
            if l >= 2:
                base = int(L_off[l - 1])
                assert pad_l[l - 1] <= W32, "fresh slab exceeds int16 idx range"
                buckets.append(("fresh", base, int(pad_l[l - 1]), fsel, 0))
            osel = in_lvl & ~e_fresh
            if l >= 3:
                max_row = int(L_off[l - 1])   # old srcs are below lvl l-1
                nw = (max_row + W32 - 1) // W32
                for w in range(nw):
                    wsel = osel & (e_srcrank >= w * W32) & (e_srcrank < (w + 1) * W32)
                    rows = min(W32, int(L_off[NL]) - w * W32)
                    # pad idx must point at an already-written row: window 0
                    # starts in the never-written level-0 slab.
                    padidx = int(L_off[1]) if w == 0 else 0
                    buckets.append(("old", w * W32, rows, wsel, padidx))
            fresh_chunks = 0
            old_chunks = 0
            for phase, base, rows, sel, padidx in buckets:
                percore = []
                for c in range(NC):
                    es = np.where(sel & (e_core == c))[0]
                    es = es[np.argsort(e_wl[es], kind="stable")]
                    percore.append(es)
                nmax = max(len(x) for x in percore)
                if nmax == 0:
                    continue
                n = ((nmax + P - 1) // P) * P
                nch = n // P
                # idx block (same wrapped layout for every core? no - idxs
                # differ per core; build per core below)
                iv = np.zeros((NC, n), np.int64)
                for c in range(NC):
                    es = percore[c]
                    iv[c, :len(es)] = e_srcrank[es] - base
                    iv[c, len(es):] = padidx
                assert iv.min() >= 0 and iv.max() < min(rows, W32)
                ch0 = fresh_chunks if phase == "fresh" else old_chunks
                info["instrs"].append({
                    "phase": phase, "base": base, "rows": rows, "n": n,
                    "icol": icol, "chunk0": ch0,
                })
                idx_cols.append(iv)
                icol += n // 16
                # pairs: for each chunk, union over cores of touched blocks
                for ch in range(nch):
                    sl = slice(ch * P, (ch + 1) * P)
                    blocks = set()
                    for c in range(NC):
                        es = percore[c][sl]
                        blocks.update(np.unique(e_wl[es] // P).tolist())
                    for b in sorted(blocks):
                        pcol = len(rank_cols[0])
                        for c in range(NC):
                            es = percore[c][sl]
                            rv = np.full(P, -1.0, np.float32)
                            wl = e_wl[es]
                            m = (wl // P) == b
                            rv[:len(es)][m] = (wl[m] - b * P).astype(np.float32)
                            rank_cols[c].append(rv)
                        info["pairs"].append(
                            (phase, ch0 + ch, int(b) // 4, int(b) % 4, pcol))
                if phase == "fresh":
                    fresh_chunks += nch
                else:
                    old_chunks += nch
            info["nfresh_chunks"] = fresh_chunks
            info["nold_chunks"] = old_chunks
            # last matmul per psum group in EMISSION order: old pairs are
            # emitted (at level l-1) before fresh pairs (at level l)
            for want in ("old", "fresh"):
                for phase, ch, grp, big, pcol in info["pairs"]:
                    if phase == want:
                        info["last"][grp] = (phase, pcol)
        levels.append(info)

    ICOLS = max(icol, 1)
    idxs = np.zeros((NC, P, ICOLS), np.int16)
    col = 0
    bi = 0
    for l in range(NL):
        for ins in levels[l]["instrs"]:
            iv = idx_cols[bi]
            bi += 1
            n = iv.shape[1]
            cols = n // 16
            for c in range(NC):
                idxs[c][:, ins["icol"]:ins["icol"] + cols] = _wrap_idx(iv[c])
            col += cols
    NPAIR = max(len(rank_cols[0]), 1)
    ranks = np.full((NC, P, NPAIR), -1.0, np.float32)
    for c in range(NC):
        if rank_cols[c]:
            ranks[c, :, :len(rank_cols[c])] = np.stack(rank_cols[c], axis=1)

    return {
        "N": N, "NL": NL, "n_l": n_l, "pad": pad_l, "L_off": L_off,
        "Vc": Vc, "Voff": Voff, "nblk": nblk, "sumVc": sumVc,
        "ICOLS": ICOLS, "NPAIR": NPAIR,
        "levels": levels, "idxs": idxs, "ranks": ranks,
        "n0row": n0row, "degrow": degrow, "node_of_rank": node_of_rank,
    }


def _prep_weights(inp):
    f64 = np.float64
    W = inp["aggr_w"].astype(f64)
    b = inp["aggr_b"].astype(f64)
    h0 = (inp["emd_w"][:, 0] + inp["emd_b"]).astype(f64)
    wih = inp["gru_wih"].astype(f64)
    whh = inp["gru_whh"].astype(f64)
    bih = inp["gru_bih"].astype(f64)
    bhh = inp["gru_bhh"].astype(f64)
    u = np.linalg.solve(W, b)
    assert np.abs(W @ u - b).max() < 1e-6
    ghc = whh @ h0 + bhh
    hr_c, hz_c, hn_c = ghc[:D], ghc[D:2 * D], ghc[2 * D:]
    bih_r, bih_z, bih_n = bih[:D], bih[D:2 * D], bih[2 * D:]
    WgT = [(wih[g * D:(g + 1) * D] @ W).T for g in range(3)]

    W1 = inp["w1"].astype(f64)  # [256, 128]
    W2 = inp["w2"].astype(f64)  # [256, 256]
    w3 = inp["w3"].astype(f64)  # [1, 256]
    assert W1.shape[0] == 256

    bf16 = _bf16()
    blocks = [
        WgT[0], WgT[1], WgT[2], np.diag(hn_c),
        W1[0:128, :].T, W1[128:256, :].T,
        W2[0:128, 0:128].T, W2[0:128, 128:256].T,
        W2[128:256, 0:128].T, W2[128:256, 128:256].T,
        np.eye(128), np.tile(np.arange(128, dtype=f64)[None, :], (128, 1)),
        np.concatenate([w3[0, 0:128, None], w3[0, 128:256, None],
                        np.zeros((128, 126))], axis=1),
    ]
    wmat = np.concatenate(blocks, axis=1).astype(bf16)  # [128, 13*128] bf16

    vcols = np.stack([
        h0,                      # 0: h0 column
        bih_r + hr_c,            # 1: sigmoid bias for r
        -(bih_z + hz_c),         # 2: sigmoid bias for z' (scale = -1)
        bih_n,                   # 3: tanh bias for n
        inp["b1"].astype(f64)[0:128],    # 4
        inp["b1"].astype(f64)[128:256],  # 5
        inp["b2"].astype(f64)[0:128],    # 6
        inp["b2"].astype(f64)[128:256],  # 7
        np.full(128, inp["b3"].astype(f64)[0]),  # 8: b3 (row 0 used)
    ], axis=1).astype(np.float32)  # [128, 9] fp32 (activation biases + h0)

    vrow = np.zeros((1, 256), np.float32)
    vrow[0, :128] = h0.astype(np.float32)
    vrow[0, 128:] = u.astype(np.float32)
    return wmat, vcols, vrow.astype(bf16)


# ---------------------------------------------------------------------------
# Bass program
# ---------------------------------------------------------------------------

WM = {name: i for i, name in enumerate(
    ["WgT_r", "WgT_z", "WgT_n", "diag_hn", "W1Ta", "W1Tb",
     "W2_k0m0", "W2_k1m0", "W2_k0m1", "W2_k1m1", "ident", "iota", "w3c"])}
VC = {name: i for i, name in enumerate(
    ["h0", "bias_r", "nbias_z", "bias_n", "b1a", "b1b", "b2a", "b2b", "b3"])}


def _build(sched, reps=1):
    import concourse.bacc as bacc
    import concourse.tile as tile
    from concourse import bass, mybir, library_config

    f32 = mybir.dt.float32
    bf = mybir.dt.bfloat16
    i16 = mybir.dt.int16
    AF = mybir.ActivationFunctionType
    OP = mybir.AluOpType
    NL = sched["NL"]
    L_off = sched["L_off"]
    Vc = sched["Vc"]
    Voff = sched["Voff"]
    pad = sched["pad"]
    ICOLS = sched["ICOLS"]
    NPAIR = sched["NPAIR"]
    sumVc = sched["sumVc"]
    NpadTot = int(L_off[NL])
    RG = [list(range(NC))]

    nc = bacc.Bacc("TRN2", target_bir_lowering=False, debug=False,
                   enable_asserts=False, num_devices=NC)

    wmat_d = nc.dram_tensor("wmat", [P, P * len(WM)], bf, kind="ExternalInput")
    vcols_d = nc.dram_tensor("vcols", [P, len(VC)], f32, kind="ExternalInput")
    vrow_d = nc.dram_tensor("vrow", [1, 256], bf, kind="ExternalInput")
    n0_d = nc.dram_tensor("n0row", [1, sumVc], bf, kind="ExternalInput")
    deg_d = nc.dram_tensor("degrow", [1, sumVc], bf, kind="ExternalInput")
    idx_d = nc.dram_tensor("idxs", [P, ICOLS], i16, kind="ExternalInput")
    rnk_d = nc.dram_tensor("ranks", [P, NPAIR], bf, kind="ExternalInput")
    pred_d = nc.dram_tensor("pred", [sumVc], f32, kind="ExternalOutput")
    h_store = nc.dram_tensor("h_store", [NpadTot, D], bf, kind="Internal",
                             addr_space="Shared")
    ag_in = [nc.dram_tensor(f"ag_in{i}", [int(Vc.max()), D], bf, kind="Internal")
             for i in range(2)]

    with tile.TileContext(nc) as tc:
        nc.gpsimd.load_library(library_config.mlp)
        cpool = tc.alloc_tile_pool(name="const", bufs=1)
        spool = tc.alloc_tile_pool(name="sbuf", bufs=2)
        gpool = tc.alloc_tile_pool(name="gath", bufs=2)
        hpool = tc.alloc_tile_pool(name="hnew", bufs=6)
        ppool = tc.alloc_tile_pool(name="psS", bufs=3, space="PSUM")
        qpool = tc.alloc_tile_pool(name="psG", bufs=3, space="PSUM")
        tpool = tc.alloc_tile_pool(name="psT", bufs=1, space="PSUM")
        rpool = tc.alloc_tile_pool(name="psP", bufs=1, space="PSUM")

        # ---- load constants ----
        wm = cpool.tile([P, P * len(WM)], bf, tag="wm")
        nc.sync.dma_start(out=wm[:], in_=wmat_d[:])
        vc = cpool.tile([P, len(VC)], f32, tag="vc")
        nc.sync.dma_start(out=vc[:], in_=vcols_d[:])
        vr = cpool.tile([1, 256], bf, tag="vr")
        nc.sync.dma_start(out=vr[:], in_=vrow_d[:])
        idxs = cpool.tile([P, ICOLS], i16, tag="idxs")
        nc.sync.dma_start(out=idxs[:], in_=idx_d[:])
        rnks = cpool.tile([P, NPAIR], bf, tag="rnks")
        nc.sync.dma_start(out=rnks[:], in_=rnk_d[:])

        def wmb(name):
            return wm[:, WM[name] * P:(WM[name] + 1) * P]

        def vcc(name):
            return vc[:, VC[name]:VC[name] + 1]

        h0b = cpool.tile([P, GW], bf, tag="h0b")  # h0 broadcast along free
        nc.vector.tensor_copy(out=h0b[:], in_=vcc("h0").to_broadcast([P, GW]))

        # ---- per-level state ----
        S_ps = [None] * NL         # list of psum tiles per level (by grp)
        last_ag = [None]           # most recent AllGather instruction

        def grp_widths(l):
            ws = []
            v = int(Vc[l])
            while v > 0:
                ws.append(min(GW, v))
                v -= GW
            return ws

        def emit_gathers(l, phase):
            """dma_gather(s) for one phase of level l."""
            if l >= NL:
                return None
            info = sched["levels"][l]
            nch = info["nfresh_chunks" if phase == "fresh" else "nold_chunks"]
            if nch == 0:
                return None
            hg = gpool.tile([P, nch * D], bf, tag="hg_" + phase)
            for ins in info["instrs"]:
                if ins["phase"] != phase:
                    continue
                n = ins["n"]
                c0 = ins["chunk0"]
                gi = nc.gpsimd.dma_gather(
                    out_ap=hg[:, c0 * D:(c0 + n // P) * D].rearrange(
                        "p (k d) -> p k d", d=D),
                    in_ap=h_store[ins["base"]:ins["base"] + ins["rows"], :],
                    idxs_ap=idxs[:, ins["icol"]:ins["icol"] + n // 16],
                    num_idxs=n,
                    num_idxs_reg=n,
                    elem_size=D,
                )
                # the windowed read of h_store races the AllGathers unless
                # pinned by hand (DRAM regions aren't shadow-tracked)
                if last_ag[0] is not None:
                    tile.add_dep_helper(gi.ins, last_ag[0].ins, sync=True,
                                        reason="gather reads AllGather output")
            return hg

        def emit_onehots(l, phase):
            info = sched["levels"][l]
            pairs = [p for p in info["pairs"] if p[0] == phase]
            if not pairs:
                return None, None
            k = len(pairs)
            p0 = pairs[0][4]
            oh = spool.tile([P, k * P], bf, tag="oh_" + phase)
            CH = 4
            for s in range(0, k, CH):
                m = min(CH, k - s)
                nc.vector.tensor_tensor(
                    out=oh[:, s * P:(s + m) * P].rearrange("p (m f) -> p m f", m=m),
                    in0=rnks[:, p0 + s:p0 + s + m][:, :, None].to_broadcast([P, m, P]),
                    in1=wmb("iota")[:, None, :].to_broadcast([P, m, P]),
                    op=OP.is_equal,
                )
            return oh, p0

        def emit_seeds(l):
            """allocate S psums for level l and seed with n0*h0 + deg*u."""
            tiles = []
            info = sched["levels"][l]
            v = int(Vc[l])
            off = int(Voff[l])
            n0r = spool.tile([1, int(Vc.max())], bf, tag="n0r")
            nc.sync.dma_start(out=n0r[0:1, :v], in_=n0_d[0:1, off:off + v])
            degr = spool.tile([1, int(Vc.max())], bf, tag="degr")
            nc.sync.dma_start(out=degr[0:1, :v], in_=deg_d[0:1, off:off + v])
            for g, w in enumerate(grp_widths(l)):
                sp = ppool.tile([P, GW], f32, tag="S", space="PSUM")
                nc.tensor.matmul(
                    out=sp[:, :w], lhsT=vr[0:1, 0:128],
                    rhs=n0r[0:1, g * GW:g * GW + w],
                    start=True, stop=False, skip_group_check=True)
                is_last = info["last"].get(g) is None
                nc.tensor.matmul(
                    out=sp[:, :w], lhsT=vr[0:1, 128:256],
                    rhs=degr[0:1, g * GW:g * GW + w],
                    start=False, stop=is_last, skip_group_check=True)
                tiles.append(sp)
            S_ps[l] = tiles

        def emit_chunks(l, phase, hg, oh, p0):
            info = sched["levels"][l]
            pairs = [p for p in info["pairs"] if p[0] == phase]
            if not pairs:
                return
            for (ph, ch, grp, big, pcol) in pairs:
                is_last = info["last"].get(grp) == (phase, pcol)
                nc.tensor.matmul(
                    out=S_ps[l][grp][:, big * P:(big + 1) * P],
                    lhsT=hg[:, ch * D:(ch + 1) * D],
                    rhs=oh[:, (pcol - p0) * P:(pcol - p0 + 1) * P],
                    start=False, stop=is_last, skip_group_check=True)

        def emit_mlp(l, g, w, rhs_sb, bcast=False):
            """MLP head for one 512-group; writes pred rows."""
            z1s = []
            for half in ("a", "b"):
                zp = qpool.tile([P, GW], f32, tag="G", space="PSUM")
                nc.tensor.matmul(out=zp[:, :w], lhsT=wmb("W1T" + half),
                                 rhs=rhs_sb[:, :w], start=True, stop=True)
                zs = spool.tile([P, GW], bf, tag="z1" + half)
                nc.scalar.activation(out=zs[:, :w], in_=zp[:, :w], func=AF.Relu,
                                     bias=vcc("b1" + half))
                z1s.append(zs)
            z2s = []
            for mi, mh in enumerate(("m0", "m1")):
                zp = qpool.tile([P, GW], f32, tag="G", space="PSUM")
                nc.tensor.matmul(out=zp[:, :w], lhsT=wmb("W2_k0" + mh),
                                 rhs=z1s[0][:, :w], start=True, stop=False)
                nc.tensor.matmul(out=zp[:, :w], lhsT=wmb("W2_k1" + mh),
                                 rhs=z1s[1][:, :w], start=False, stop=True)
                zs = spool.tile([P, GW], bf, tag="z2" + mh)
                nc.scalar.activation(out=zs[:, :w], in_=zp[:, :w], func=AF.Relu,
                                     bias=vcc("b2" + ("a" if mi == 0 else "b")))
                z2s.append(zs)
            pp = rpool.tile([1, GW], f32, tag="pred", space="PSUM")
            nc.tensor.matmul(out=pp[:, :w], lhsT=wmb("w3c")[:, 0:1],
                             rhs=z2s[0][:, :w], start=True, stop=False)
            nc.tensor.matmul(out=pp[:, :w], lhsT=wmb("w3c")[:, 1:2],
                             rhs=z2s[1][:, :w], start=False, stop=True)
            ps = spool.tile([1, GW], f32, tag="psb")
            nc.scalar.activation(out=ps[:, :w], in_=pp[:, :w], func=AF.Identity,
                                 bias=vc[0:1, VC["b3"]:VC["b3"] + 1])
            if bcast:
                pbc = spool.tile([1, GW], f32, tag="pbc")
                nc.vector.tensor_copy(out=pbc[0:1, :],
                                      in_=ps[0:1, 0:1].to_broadcast([1, GW]))
                for gg, ww in enumerate(grp_widths(l)):
                    off = int(Voff[l]) + gg * GW
                    nc.sync.dma_start(out=pred_d[off:off + ww],
                                      in_=pbc[0:1, :ww])
            else:
                off = int(Voff[l]) + g * GW
                nc.sync.dma_start(out=pred_d[off:off + w], in_=ps[0:1, :w])

        # reps>1 repeats the whole computation for wall-clock timing: the
        # computation is idempotent (h_store/pred rewritten with same values)
        for _rep in range(reps):
          # ================= level 0: one column, broadcast ==============
          # every level-0 node keeps h = h0, so pred is a single scalar
          emit_mlp(0, 0, 1, h0b, bcast=True)

          # seeds + (no old/fresh chunks possible) for level 1
          emit_seeds(1)
          Old_sb = [None] * (NL + 1)
          OH = {}  # one-hots, generated one iteration ahead

          def prefetch_oh(t):
              if t < NL:
                  OH[(t, "f")] = emit_onehots(t, "fresh")
                  if t + 1 < NL:
                      OH[(t + 1, "o")] = emit_onehots(t + 1, "old")

          prefetch_oh(1)

          # ================= levels 1..NL-1 =================
          for l in range(1, NL):
            widths = grp_widths(l)

            # fresh gather + chunks for this level
            hg_f = emit_gathers(l, "fresh")
            oh_f, p0_f = OH.get((l, "f"), (None, None))
            if hg_f is not None:
                emit_chunks(l, "fresh", hg_f, oh_f, p0_f)

            # old gather for next level: its sources are at levels <= l-1,
            # i.e. rows below L_off[l], so it overlaps this level's AllGather
            if l + 1 < NL:
                Old_sb[l + 1] = emit_gathers(l + 1, "old")

            # GRU per group
            hnew = []
            for g, w in enumerate(widths):
                veng = nc.vector
                ssb = spool.tile([P, GW], bf, tag="Ssb")
                nc.vector.tensor_copy(out=ssb[:, :w], in_=S_ps[l][g][:, :w])

                gr = qpool.tile([P, GW], f32, tag="G", space="PSUM")
                nc.tensor.matmul(out=gr[:, :w], lhsT=wmb("WgT_r"),
                                 rhs=ssb[:, :w], start=True, stop=True)
                gz = qpool.tile([P, GW], f32, tag="G", space="PSUM")
                nc.tensor.matmul(out=gz[:, :w], lhsT=wmb("WgT_z"),
                                 rhs=ssb[:, :w], start=True, stop=True)
                gn = qpool.tile([P, GW], f32, tag="G", space="PSUM")
                nc.tensor.matmul(out=gn[:, :w], lhsT=wmb("WgT_n"),
                                 rhs=ssb[:, :w], start=True, stop=False)

                rsb = spool.tile([P, GW], bf, tag="rsb")
                nc.scalar.activation(out=rsb[:, :w], in_=gr[:, :w],
                                     func=AF.Sigmoid, bias=vcc("bias_r"))
                zsb = spool.tile([P, GW], bf, tag="zsb")
                nc.scalar.activation(out=zsb[:, :w], in_=gz[:, :w],
                                     func=AF.Sigmoid, bias=vcc("nbias_z"),
                                     scale=-1.0)
                nc.tensor.matmul(out=gn[:, :w], lhsT=wmb("diag_hn"),
                                 rhs=rsb[:, :w], start=False, stop=True)
                nsb = spool.tile([P, GW], bf, tag="nsb")
                nc.scalar.activation(out=nsb[:, :w], in_=gn[:, :w],
                                     func=AF.Tanh, bias=vcc("bias_n"))

                t3 = spool.tile([P, GW], bf, tag="t3")
                veng.tensor_scalar(out=t3[:, :w], in0=nsb[:, :w],
                                   scalar1=vcc("h0"), scalar2=None,
                                   op0=OP.subtract)
                t4 = spool.tile([P, GW], bf, tag="t4")
                veng.tensor_tensor(out=t4[:, :w], in0=t3[:, :w],
                                   in1=zsb[:, :w], op=OP.mult)
                hn = hpool.tile([P, GW], bf, tag="hnew")
                veng.tensor_scalar(out=hn[:, :w], in0=t4[:, :w],
                                   scalar1=vcc("h0"), scalar2=None,
                                   op0=OP.add)
                hnew.append(hn)

                # stage this group for the AllGather as soon as its h_new
                # is ready (transpose to node-major, copy, DMA)
                if l < NL - 1:
                    agt = ag_in[l % 2]
                    tp = tpool.tile([P, GW], bf, tag="tp", space="PSUM")
                    nb = w // P
                    for b in range(nb):
                        nc.tensor.transpose(
                            out=tp[:, b * P:(b + 1) * P],
                            in_=hn[:, b * P:(b + 1) * P],
                            identity=wmb("ident"))
                    tps = spool.tile([P, GW], bf, tag="tps")
                    nc.vector.tensor_copy(out=tps[:, :w], in_=tp[:, :w])
                    for b in range(nb):
                        row = g * GW + b * P
                        nc.sync.dma_start(out=agt[row:row + P, :],
                                          in_=tps[:, b * P:(b + 1) * P])

            if l < NL - 1:
                cc = nc.gpsimd.collective_compute(
                    "AllGather", mybir.AluOpType.bypass,
                    replica_groups=RG,
                    ins=[agt[0:int(Vc[l]), :].opt()],
                    outs=[h_store[int(L_off[l]):int(L_off[l]) + int(pad[l]), :].opt()],
                )
                last_ag[0] = cc

            # MLP head for this level (fills the AllGather latency)
            for g, w in enumerate(widths):
                emit_mlp(l, g, w, hnew[g])

            # seeds + old chunks for the next level (also fill the AllGather)
            if l + 1 < NL:
                emit_seeds(l + 1)
                oh_o, p0_o = OH.get((l + 1, "o"), (None, None))
                if Old_sb[l + 1] is not None:
                    emit_chunks(l + 1, "old", Old_sb[l + 1], oh_o, p0_o)

            prefetch_oh(l + 1)

        for pl in (rpool, tpool, qpool, ppool, hpool, gpool, spool, cpool):
            pl.release()

    nc.compile()
    return nc


# ---------------------------------------------------------------------------
# Entry point
# ---------------------------------------------------------------------------

def _run(inputs, trace=False, reps=1):
    from concourse.bass_utils import run_bass_kernel_spmd

    inputs = {k: np.asarray(v) for k, v in inputs.items()}
    bf16 = _bf16()
    fl = np.asarray(inputs["forward_level"])
    num_levels = int(fl.max()) + 1
    sched = _preprocess(fl, inputs["edge_index"], num_levels)
    wmat, vcols, vrow = _prep_weights(inputs)

    key = (sched["N"], sched["ICOLS"], sched["NPAIR"], sched["sumVc"], reps,
           tuple(int(x) for x in sched["Vc"]),
           tuple((len(i["instrs"]), len(i["pairs"]))
                 for i in sched["levels"]))
    if key not in _COMPILED:
        _COMPILED[key] = _build(sched, reps=reps)
    nc = _COMPILED[key]

    in_maps = []
    for c in range(NC):
        in_maps.append({
            "wmat": wmat, "vcols": vcols, "vrow": vrow,
            "n0row": sched["n0row"][c][None, :].astype(bf16),
            "degrow": sched["degrow"][c][None, :].astype(bf16),
            "idxs": sched["idxs"][c],
            "ranks": sched["ranks"][c].astype(bf16),
        })

    res = run_bass_kernel_spmd(nc, in_maps, core_ids=list(range(NC)),
                               trace=trace)

    NL = sched["NL"]
    L_off, Vc, Voff = sched["L_off"], sched["Vc"], sched["Voff"]
    node_of_rank = sched["node_of_rank"]
    out = np.zeros(sched["N"], np.float32)
    for c in range(NC):
        oc = res.results[c]["pred"]
        for l in range(NL):
            gr = int(L_off[l]) + c * int(Vc[l]) + np.arange(int(Vc[l]))
            nd = node_of_rank[gr]
            m = nd >= 0
            out[nd[m]] = oc[int(Voff[l]):int(Voff[l]) + int(Vc[l])][m]
    return out[:, None], res


def kernel(**inputs):
    out, _ = _run(inputs, trace=False)
    return out


# revision 68
# speedup vs baseline: 1.1311x; 1.1238x over previous
"""DeepSAT GNN message-passing kernel for 8 Trainium2 NeuronCores.

Algorithm notes (validated numerically against the reference):
  - Every node is updated exactly once, at step l = forward_level (levels
    1..19; level-0 nodes keep h0 forever). At update time the node's own
    hidden state is still h0, so the GRU "hidden side" gates are constant
    vectors computable on the host.
  - msg_i = W @ (S_i + n0_i*h0) + deg_i*b, where S_i sums h over "active"
    in-edges (source level in [1, level_i)), n0_i counts inactive in-edges
    and deg_i all in-edges. With u = W^-1 b this folds to
    msg_i = W @ S'_i,  S'_i = S_i + n0_i*h0 + deg_i*u, so the per-gate
    input is  gi_g = (wih_g @ W) @ S'_i + bih_g  -- one fused matmul.
  - Nodes are stored level-sorted ("rank" order): per-level writes are
    contiguous, per-level ownership is an even 8-way split, and the
    AllGather of each level's new h lands in place.

Perf design (v2):
  - All matmuls/h-storage in bf16 (PE runs 1 cycle/row vs 4 for fp32);
    PSUM accumulation stays fp32.
  - Edge gathers use gpsimd.dma_gather (one SWDGE instruction per up to
    ~2-3k edges) instead of one indirect DMA per 128 edges: SWDGE fixed
    overhead is ~1us per instruction and was the previous bottleneck.
    dma_gather indices are int16, so "old" gathers (sources anywhere
    below the current level) are windowed into 32768-row slabs of
    h_store; "fresh" gathers (sources in level l-1 only) use the l-1
    slab directly.
  - Edges are sorted by destination slot so each 128-edge chunk spans
    few 128-slot blocks (~85% fill vs 44% before): fewer descriptors
    and fewer one-hot segment-sum matmuls.

Device schedule per level l (SPMD on 8 cores):
  dma_gather h[src] for this level's "fresh" edges (src level == l-1),
  segment-sum via one-hot matmuls into PSUM (seeded with the n0/deg
  terms), fused GRU, PE-transpose, DMA to the AllGather input, AllGather
  into the replicated h_store, then (overlapping the collective) the MLP
  head for this level plus the next level's "old" gathers/chunks, whose
  windows only read rows below this level's slab.
"""

import sys
import numpy as np

sys.path.insert(0, "/opt/trn_rl_repo")

P = 128
D = 128
NC = 8
GW = 512  # psum group width (one bank of fp32)
W32 = 32768  # dma_gather int16 index window

_COMPILED = {}


def _bf16():
    import ml_dtypes
    return ml_dtypes.bfloat16


# ---------------------------------------------------------------------------
# Host-side preprocessing
# ---------------------------------------------------------------------------

def _wrap_idx(vals):
    """int16 idx layout: j at [j%16, j//16], replicated to 8x16 partitions."""
    n = len(vals)
    cols = (n + 15) // 16
    t = np.zeros((P, cols), np.int16)
    for k in range(8):
        t[16 * k + (np.arange(n) % 16), np.arange(n) // 16] = vals
    return t


def _preprocess(forward_level, edge_index, num_levels):
    fl = np.asarray(forward_level).astype(np.int64)
    ei = np.asarray(edge_index).astype(np.int64)
    src, dst = ei[0], ei[1]
    N = fl.shape[0]
    NL = num_levels

    # --- rank space: nodes sorted by level, each level padded to NC*P ---
    n_l = np.bincount(fl, minlength=NL).astype(np.int64)
    pad_l = ((n_l + NC * P - 1) // (NC * P)) * (NC * P)
    pad_l = np.maximum(pad_l, NC * P)
    L_off = np.zeros(NL + 1, np.int64)
    L_off[1:] = np.cumsum(pad_l)
    Vc = (pad_l // NC).astype(np.int64)          # per-core nodes per level
    Voff = np.zeros(NL + 1, np.int64)
    Voff[1:] = np.cumsum(Vc)                     # per-core rank-space offsets
    nblk = (Vc // P).astype(np.int64)

    order = np.argsort(fl, kind="stable")
    starts_real = np.zeros(NL + 1, np.int64)
    starts_real[1:] = np.cumsum(n_l)

    lv_s, lv_d = fl[src], fl[dst]
    act = (lv_s >= 1) & (lv_s < lv_d)
    deg = np.bincount(dst, minlength=N).astype(np.float64)
    n0 = np.bincount(dst[~act], minlength=N).astype(np.float64)

    # "sourced" nodes (>=1 active out-edge) take the FIRST slots of each
    # core's span, so the AllGather only has to replicate a prefix of each
    # level (h of unsourced nodes is never gathered). Source-side reads go
    # through a compacted h_store: level l occupies NC*K[l] rows.
    sourced = np.zeros(N, bool)
    sourced[src[np.where(act)[0]]] = True
    rank = np.empty(N, np.int64)        # dst/slot space (full, as before)
    csrc = np.full(N, -1, np.int64)     # compacted source space
    K = np.zeros(NL, np.int64)
    for l in range(NL):
        nodes_l = order[starts_real[l]:starts_real[l + 1]]
        c_of = np.arange(len(nodes_l)) // int(Vc[l])
        kmax = 0
        for c in range(NC):
            kmax = max(kmax, int(sourced[nodes_l[c_of == c]].sum()))
        K[l] = min(((kmax + P - 1) // P) * P, int(Vc[l])) if kmax else 0
    CL_off = np.zeros(NL + 1, np.int64)
    CL_off[1:] = np.cumsum(K * NC)
    for l in range(NL):
        nodes_l = order[starts_real[l]:starts_real[l + 1]]
        c_of = np.arange(len(nodes_l)) // int(Vc[l])
        for c in range(NC):
            mem = nodes_l[c_of == c]
            sm = sourced[mem]
            slot_order = np.concatenate([mem[sm], mem[~sm]])
            rank[slot_order] = (L_off[l] + c * Vc[l]
                                + np.arange(len(mem), dtype=np.int64))
            ns = int(sm.sum())
            csrc[mem[sm]] = (CL_off[l] + c * K[l]
                             + np.arange(ns, dtype=np.int64))

    node_of_rank = np.full(L_off[NL], -1, np.int64)
    node_of_rank[rank] = np.arange(N, dtype=np.int64)

    sumVc = int(Voff[NL])
    n0row = np.zeros((NC, sumVc), np.float32)
    degrow = np.zeros((NC, sumVc), np.float32)
    for c in range(NC):
        grs = []
        for l in range(NL):
            grs.append(L_off[l] + c * Vc[l] + np.arange(Vc[l]))
        gr = np.concatenate(grs)
        nd = node_of_rank[gr]
        m = nd >= 0
        n0row[c, m] = n0[nd[m]]
        degrow[c, m] = deg[nd[m]]

    # --- active edge table ---
    er = np.where(act)[0]
    e_lvl = lv_d[er]
    e_srcrank = csrc[src[er]].astype(np.int64)
    assert e_srcrank.min() >= 0
    e_dstrank = rank[dst[er]].astype(np.int64)
    e_local = e_dstrank - L_off[e_lvl]
    e_core = e_local // Vc[e_lvl]
    e_wl = e_local % Vc[e_lvl]          # slot within core's level span
    e_fresh = lv_s[er] == (e_lvl - 1)

    # Per (level, phase[, window]) gather instructions; edges sorted by dst
    # slot so chunks span few blocks. All counts are the max over cores so
    # the SPMD program is identical everywhere.
    idx_cols = []                # list of [128, n/16] int16 blocks
    icol = 0
    rank_cols = [[] for _ in range(NC)]  # per pair: [128] f32 block-rel slots
    levels = []
    for l in range(NL):
        info = {"instrs": [], "pairs": [], "last": {},
                "ngrp": (int(Vc[l]) + GW - 1) // GW,
                "fresh_chunk0": 0, "old_chunk0": 0,
                "nfresh_chunks": 0, "nold_chunks": 0}
        if l >= 1:
            in_lvl = e_lvl == l
            # bucket list: ("fresh", base, rows, sel) or ("old", w, ...)
            buckets = []
            fsel = in_lvl & e_fresh
            if l >= 2 and K[l - 1] > 0:
                base = int(CL_off[l - 1])
                rows = int(NC * K[l - 1])
                assert rows <= W32, "fresh slab exceeds int16 idx range"
                buckets.append(("fresh", base, rows, fsel, 0))
            osel = in_lvl & ~e_fresh
            if l >= 3:
                max_row = int(CL_off[l - 1])  # old srcs are below lvl l-1
                nw = (max_row + W32 - 1) // W32
                for w in range(nw):
                    wsel = osel & (e_srcrank >= w * W32) & (e_srcrank < (w + 1) * W32)
                    rows = min(W32, int(CL_off[NL]) - w * W32)
                    # pad idx 0 = the window's first row, always inside an
                    # already-AllGathered compact slab (no level-0 slab here)
                    buckets.append(("old", w * W32, rows, wsel, 0))
            fresh_chunks = 0
            old_chunks = 0
            for phase, base, rows, sel, padidx in buckets:
                percore = []
                for c in range(NC):
                    es = np.where(sel & (e_core == c))[0]
                    es = es[np.argsort(e_wl[es], kind="stable")]
                    percore.append(es)
                nmax = max(len(x) for x in percore)
                if nmax == 0:
                    continue
                n = ((nmax + P - 1) // P) * P
                nch = n // P
                # idx block (same wrapped layout for every core? no - idxs
                # differ per core; build per core below)
                iv = np.zeros((NC, n), np.int64)
                for c in range(NC):
                    es = percore[c]
                    iv[c, :len(es)] = e_srcrank[es] - base
                    iv[c, len(es):] = padidx
                assert iv.min() >= 0 and iv.max() < min(rows, W32)
                ch0 = fresh_chunks if phase == "fresh" else old_chunks
                info["instrs"].append({
                    "phase": phase, "base": base, "rows": rows, "n": n,
                    "icol": icol, "chunk0": ch0,
                })
                idx_cols.append(iv)
                icol += n // 16
                # pairs: for each chunk, union over cores of touched blocks
                for ch in range(nch):
                    sl = slice(ch * P, (ch + 1) * P)
                    blocks = set()
                    for c in range(NC):
                        es = percore[c][sl]
                        blocks.update(np.unique(e_wl[es] // P).tolist())
                    for b in sorted(blocks):
                        pcol = len(rank_cols[0])
                        for c in range(NC):
                            es = percore[c][sl]
                            rv = np.full(P, -1.0, np.float32)
                            wl = e_wl[es]
                            m = (wl // P) == b
                            rv[:len(es)][m] = (wl[m] - b * P).astype(np.float32)
                            rank_cols[c].append(rv)
                        info["pairs"].append(
                            (phase, ch0 + ch, int(b) // 4, int(b) % 4, pcol))
                if phase == "fresh":
                    fresh_chunks += nch
                else:
                    old_chunks += nch
            info["nfresh_chunks"] = fresh_chunks
            info["nold_chunks"] = old_chunks
            # last matmul per psum group in EMISSION order: old pairs are
            # emitted (at level l-1) before fresh pairs (at level l)
            for want in ("old", "fresh"):
                for phase, ch, grp, big, pcol in info["pairs"]:
                    if phase == want:
                        info["last"][grp] = (phase, pcol)
        levels.append(info)

    ICOLS = max(icol, 1)
    idxs = np.zeros((NC, P, ICOLS), np.int16)
    col = 0
    bi = 0
    for l in range(NL):
        for ins in levels[l]["instrs"]:
            iv = idx_cols[bi]
            bi += 1
            n = iv.shape[1]
            cols = n // 16
            for c in range(NC):
                idxs[c][:, ins["icol"]:ins["icol"] + cols] = _wrap_idx(iv[c])
            col += cols
    NPAIR = max(len(rank_cols[0]), 1)
    ranks = np.full((NC, P, NPAIR), -1.0, np.float32)
    for c in range(NC):
        if rank_cols[c]:
            ranks[c, :, :len(rank_cols[c])] = np.stack(rank_cols[c], axis=1)

    return {
        "N": N, "NL": NL, "n_l": n_l, "pad": pad_l, "L_off": L_off,
        "Vc": Vc, "Voff": Voff, "nblk": nblk, "sumVc": sumVc,
        "ICOLS": ICOLS, "NPAIR": NPAIR, "K": K, "CL_off": CL_off,
        "levels": levels, "idxs": idxs, "ranks": ranks,
        "n0row": n0row, "degrow": degrow, "node_of_rank": node_of_rank,
    }


def _prep_weights(inp):
    f64 = np.float64
    W = inp["aggr_w"].astype(f64)
    b = inp["aggr_b"].astype(f64)
    h0 = (inp["emd_w"][:, 0] + inp["emd_b"]).astype(f64)
    wih = inp["gru_wih"].astype(f64)
    whh = inp["gru_whh"].astype(f64)
    bih = inp["gru_bih"].astype(f64)
    bhh = inp["gru_bhh"].astype(f64)
    u = np.linalg.solve(W, b)
    assert np.abs(W @ u - b).max() < 1e-6
    ghc = whh @ h0 + bhh
    hr_c, hz_c, hn_c = ghc[:D], ghc[D:2 * D], ghc[2 * D:]
    bih_r, bih_z, bih_n = bih[:D], bih[D:2 * D], bih[2 * D:]
    WgT = [(wih[g * D:(g + 1) * D] @ W).T for g in range(3)]

    W1 = inp["w1"].astype(f64)  # [256, 128]
    W2 = inp["w2"].astype(f64)  # [256, 256]
    w3 = inp["w3"].astype(f64)  # [1, 256]
    assert W1.shape[0] == 256

    bf16 = _bf16()
    blocks = [
        WgT[0], WgT[1], WgT[2], np.diag(hn_c),
        W1[0:128, :].T, W1[128:256, :].T,
        W2[0:128, 0:128].T, W2[0:128, 128:256].T,
        W2[128:256, 0:128].T, W2[128:256, 128:256].T,
        np.eye(128), np.tile(np.arange(128, dtype=f64)[None, :], (128, 1)),
        np.concatenate([w3[0, 0:128, None], w3[0, 128:256, None],
                        np.zeros((128, 126))], axis=1),
    ]
    wmat = np.concatenate(blocks, axis=1).astype(bf16)  # [128, 13*128] bf16

    vcols = np.stack([
        h0,                      # 0: h0 column
        bih_r + hr_c,            # 1: sigmoid bias for r
        -(bih_z + hz_c),         # 2: sigmoid bias for z' (scale = -1)
        bih_n,                   # 3: tanh bias for n
        inp["b1"].astype(f64)[0:128],    # 4
        inp["b1"].astype(f64)[128:256],  # 5
        inp["b2"].astype(f64)[0:128],    # 6
        inp["b2"].astype(f64)[128:256],  # 7
        np.full(128, inp["b3"].astype(f64)[0]),  # 8: b3 (row 0 used)
    ], axis=1).astype(np.float32)  # [128, 9] fp32 (activation biases + h0)

    vrow = np.zeros((1, 256), np.float32)
    vrow[0, :128] = h0.astype(np.float32)
    vrow[0, 128:] = u.astype(np.float32)
    return wmat, vcols, vrow.astype(bf16)


# ---------------------------------------------------------------------------
# Bass program
# ---------------------------------------------------------------------------

WM = {name: i for i, name in enumerate(
    ["WgT_r", "WgT_z", "WgT_n", "diag_hn", "W1Ta", "W1Tb",
     "W2_k0m0", "W2_k1m0", "W2_k0m1", "W2_k1m1", "ident", "iota", "w3c"])}
VC = {name: i for i, name in enumerate(
    ["h0", "bias_r", "nbias_z", "bias_n", "b1a", "b1b", "b2a", "b2b", "b3"])}


def _build(sched, reps=1):
    import concourse.bacc as bacc
    import concourse.tile as tile
    from concourse import bass, mybir, library_config

    f32 = mybir.dt.float32
    bf = mybir.dt.bfloat16
    i16 = mybir.dt.int16
    AF = mybir.ActivationFunctionType
    OP = mybir.AluOpType
    NL = sched["NL"]
    L_off = sched["L_off"]
    Vc = sched["Vc"]
    Voff = sched["Voff"]
    pad = sched["pad"]
    ICOLS = sched["ICOLS"]
    NPAIR = sched["NPAIR"]
    sumVc = sched["sumVc"]
    K = sched["K"]
    CL_off = sched["CL_off"]
    NpadTot = max(int(CL_off[NL]), 1)
    RG = [list(range(NC))]

    nc = bacc.Bacc("TRN2", target_bir_lowering=False, debug=False,
                   enable_asserts=False, num_devices=NC)

    wmat_d = nc.dram_tensor("wmat", [P, P * len(WM)], bf, kind="ExternalInput")
    vcols_d = nc.dram_tensor("vcols", [P, len(VC)], f32, kind="ExternalInput")
    vrow_d = nc.dram_tensor("vrow", [1, 256], bf, kind="ExternalInput")
    n0_d = nc.dram_tensor("n0row", [1, sumVc], bf, kind="ExternalInput")
    deg_d = nc.dram_tensor("degrow", [1, sumVc], bf, kind="ExternalInput")
    idx_d = nc.dram_tensor("idxs", [P, ICOLS], i16, kind="ExternalInput")
    rnk_d = nc.dram_tensor("ranks", [P, NPAIR], bf, kind="ExternalInput")
    pred_d = nc.dram_tensor("pred", [sumVc], f32, kind="ExternalOutput")
    h_store = nc.dram_tensor("h_store", [NpadTot, D], bf, kind="Internal",
                             addr_space="Shared")
    ag_in = [nc.dram_tensor(f"ag_in{i}", [int(Vc.max()), D], bf, kind="Internal")
             for i in range(2)]

    with tile.TileContext(nc) as tc:
        nc.gpsimd.load_library(library_config.mlp)
        cpool = tc.alloc_tile_pool(name="const", bufs=1)
        spool = tc.alloc_tile_pool(name="sbuf", bufs=2)
        gpool = tc.alloc_tile_pool(name="gath", bufs=2)
        hpool = tc.alloc_tile_pool(name="hnew", bufs=6)
        ppool = tc.alloc_tile_pool(name="psS", bufs=3, space="PSUM")
        qpool = tc.alloc_tile_pool(name="psG", bufs=3, space="PSUM")
        tpool = tc.alloc_tile_pool(name="psT", bufs=1, space="PSUM")
        rpool = tc.alloc_tile_pool(name="psP", bufs=1, space="PSUM")

        # ---- load constants ----
        wm = cpool.tile([P, P * len(WM)], bf, tag="wm")
        nc.sync.dma_start(out=wm[:], in_=wmat_d[:])
        vc = cpool.tile([P, len(VC)], f32, tag="vc")
        nc.sync.dma_start(out=vc[:], in_=vcols_d[:])
        vr = cpool.tile([1, 256], bf, tag="vr")
        nc.sync.dma_start(out=vr[:], in_=vrow_d[:])
        idxs = cpool.tile([P, ICOLS], i16, tag="idxs")
        nc.sync.dma_start(out=idxs[:], in_=idx_d[:])
        rnks = cpool.tile([P, NPAIR], bf, tag="rnks")
        nc.sync.dma_start(out=rnks[:], in_=rnk_d[:])

        def wmb(name):
            return wm[:, WM[name] * P:(WM[name] + 1) * P]

        def vcc(name):
            return vc[:, VC[name]:VC[name] + 1]

        h0b = cpool.tile([P, GW], bf, tag="h0b")  # h0 broadcast along free
        nc.vector.tensor_copy(out=h0b[:], in_=vcc("h0").to_broadcast([P, GW]))

        # ---- per-level state ----
        S_ps = [None] * NL         # list of psum tiles per level (by grp)
        last_ag = [None]           # most recent AllGather instruction

        def grp_widths(l):
            ws = []
            v = int(Vc[l])
            while v > 0:
                ws.append(min(GW, v))
                v -= GW
            return ws

        def emit_gathers(l, phase):
            """dma_gather(s) for one phase of level l."""
            if l >= NL:
                return None
            info = sched["levels"][l]
            nch = info["nfresh_chunks" if phase == "fresh" else "nold_chunks"]
            if nch == 0:
                return None
            hg = gpool.tile([P, nch * D], bf, tag="hg_" + phase)
            for ins in info["instrs"]:
                if ins["phase"] != phase:
                    continue
                n = ins["n"]
                c0 = ins["chunk0"]
                gi = nc.gpsimd.dma_gather(
                    out_ap=hg[:, c0 * D:(c0 + n // P) * D].rearrange(
                        "p (k d) -> p k d", d=D),
                    in_ap=h_store[ins["base"]:ins["base"] + ins["rows"], :],
                    idxs_ap=idxs[:, ins["icol"]:ins["icol"] + n // 16],
                    num_idxs=n,
                    num_idxs_reg=n,
                    elem_size=D,
                )
                # the windowed read of h_store races the AllGathers unless
                # pinned by hand (DRAM regions aren't shadow-tracked)
                if last_ag[0] is not None:
                    tile.add_dep_helper(gi.ins, last_ag[0].ins, sync=True,
                                        reason="gather reads AllGather output")
            return hg

        def emit_onehots(l, phase):
            info = sched["levels"][l]
            pairs = [p for p in info["pairs"] if p[0] == phase]
            if not pairs:
                return None, None
            k = len(pairs)
            p0 = pairs[0][4]
            oh = spool.tile([P, k * P], bf, tag="oh_" + phase)
            CH = 4
            for s in range(0, k, CH):
                m = min(CH, k - s)
                nc.vector.tensor_tensor(
                    out=oh[:, s * P:(s + m) * P].rearrange("p (m f) -> p m f", m=m),
                    in0=rnks[:, p0 + s:p0 + s + m][:, :, None].to_broadcast([P, m, P]),
                    in1=wmb("iota")[:, None, :].to_broadcast([P, m, P]),
                    op=OP.is_equal,
                )
            return oh, p0

        def emit_seeds(l):
            """allocate S psums for level l and seed with n0*h0 + deg*u."""
            tiles = []
            info = sched["levels"][l]
            v = int(Vc[l])
            off = int(Voff[l])
            n0r = spool.tile([1, int(Vc.max())], bf, tag="n0r")
            nc.sync.dma_start(out=n0r[0:1, :v], in_=n0_d[0:1, off:off + v])
            degr = spool.tile([1, int(Vc.max())], bf, tag="degr")
            nc.sync.dma_start(out=degr[0:1, :v], in_=deg_d[0:1, off:off + v])
            for g, w in enumerate(grp_widths(l)):
                sp = ppool.tile([P, GW], f32, tag="S", space="PSUM")
                nc.tensor.matmul(
                    out=sp[:, :w], lhsT=vr[0:1, 0:128],
                    rhs=n0r[0:1, g * GW:g * GW + w],
                    start=True, stop=False, skip_group_check=True)
                is_last = info["last"].get(g) is None
                nc.tensor.matmul(
                    out=sp[:, :w], lhsT=vr[0:1, 128:256],
                    rhs=degr[0:1, g * GW:g * GW + w],
                    start=False, stop=is_last, skip_group_check=True)
                tiles.append(sp)
            S_ps[l] = tiles

        def emit_chunks(l, phase, hg, oh, p0):
            info = sched["levels"][l]
            pairs = [p for p in info["pairs"] if p[0] == phase]
            if not pairs:
                return
            for (ph, ch, grp, big, pcol) in pairs:
                is_last = info["last"].get(grp) == (phase, pcol)
                nc.tensor.matmul(
                    out=S_ps[l][grp][:, big * P:(big + 1) * P],
                    lhsT=hg[:, ch * D:(ch + 1) * D],
                    rhs=oh[:, (pcol - p0) * P:(pcol - p0 + 1) * P],
                    start=False, stop=is_last, skip_group_check=True)

        def emit_mlp(l, g, w, rhs_sb, bcast=False):
            """MLP head for one 512-group; writes pred rows."""
            z1s = []
            for half in ("a", "b"):
                zp = qpool.tile([P, GW], f32, tag="G", space="PSUM")
                nc.tensor.matmul(out=zp[:, :w], lhsT=wmb("W1T" + half),
                                 rhs=rhs_sb[:, :w], start=True, stop=True)
                zs = spool.tile([P, GW], bf, tag="z1" + half)
                nc.scalar.activation(out=zs[:, :w], in_=zp[:, :w], func=AF.Relu,
                                     bias=vcc("b1" + half))
                z1s.append(zs)
            z2s = []
            for mi, mh in enumerate(("m0", "m1")):
                zp = qpool.tile([P, GW], f32, tag="G", space="PSUM")
                nc.tensor.matmul(out=zp[:, :w], lhsT=wmb("W2_k0" + mh),
                                 rhs=z1s[0][:, :w], start=True, stop=False)
                nc.tensor.matmul(out=zp[:, :w], lhsT=wmb("W2_k1" + mh),
                                 rhs=z1s[1][:, :w], start=False, stop=True)
                zs = spool.tile([P, GW], bf, tag="z2" + mh)
                nc.scalar.activation(out=zs[:, :w], in_=zp[:, :w], func=AF.Relu,
                                     bias=vcc("b2" + ("a" if mi == 0 else "b")))
                z2s.append(zs)
            pp = rpool.tile([1, GW], f32, tag="pred", space="PSUM")
            nc.tensor.matmul(out=pp[:, :w], lhsT=wmb("w3c")[:, 0:1],
                             rhs=z2s[0][:, :w], start=True, stop=False)
            nc.tensor.matmul(out=pp[:, :w], lhsT=wmb("w3c")[:, 1:2],
                             rhs=z2s[1][:, :w], start=False, stop=True)
            ps = spool.tile([1, GW], f32, tag="psb")
            nc.scalar.activation(out=ps[:, :w], in_=pp[:, :w], func=AF.Identity,
                                 bias=vc[0:1, VC["b3"]:VC["b3"] + 1])
            if bcast:
                pbc = spool.tile([1, GW], f32, tag="pbc")
                nc.vector.tensor_copy(out=pbc[0:1, :],
                                      in_=ps[0:1, 0:1].to_broadcast([1, GW]))
                for gg, ww in enumerate(grp_widths(l)):
                    off = int(Voff[l]) + gg * GW
                    nc.sync.dma_start(out=pred_d[off:off + ww],
                                      in_=pbc[0:1, :ww])
            else:
                off = int(Voff[l]) + g * GW
                nc.sync.dma_start(out=pred_d[off:off + w], in_=ps[0:1, :w])

        # reps>1 repeats the whole computation for wall-clock timing: the
        # computation is idempotent (h_store/pred rewritten with same values)
        for _rep in range(reps):
          # ================= level 0: one column, broadcast ==============
          # every level-0 node keeps h = h0, so pred is a single scalar
          emit_mlp(0, 0, 1, h0b, bcast=True)

          # seeds + (no old/fresh chunks possible) for level 1
          emit_seeds(1)
          Old_sb = [None] * (NL + 1)
          OH = {}  # one-hots, generated one iteration ahead

          def prefetch_oh(t):
              if t < NL:
                  OH[(t, "f")] = emit_onehots(t, "fresh")
                  if t + 1 < NL:
                      OH[(t + 1, "o")] = emit_onehots(t + 1, "old")

          prefetch_oh(1)

          # ================= levels 1..NL-1 =================
          for l in range(1, NL):
            widths = grp_widths(l)

            # fresh gather + chunks for this level
            hg_f = emit_gathers(l, "fresh")
            oh_f, p0_f = OH.get((l, "f"), (None, None))
            if hg_f is not None:
                emit_chunks(l, "fresh", hg_f, oh_f, p0_f)

            # old gather for next level: its sources are at levels <= l-1,
            # i.e. rows below L_off[l], so it overlaps this level's AllGather
            if l + 1 < NL:
                Old_sb[l + 1] = emit_gathers(l + 1, "old")

            # GRU per group
            hnew = []
            for g, w in enumerate(widths):
                veng = nc.vector
                ssb = spool.tile([P, GW], bf, tag="Ssb")
                nc.vector.tensor_copy(out=ssb[:, :w], in_=S_ps[l][g][:, :w])

                gr = qpool.tile([P, GW], f32, tag="G", space="PSUM")
                nc.tensor.matmul(out=gr[:, :w], lhsT=wmb("WgT_r"),
                                 rhs=ssb[:, :w], start=True, stop=True)
                gz = qpool.tile([P, GW], f32, tag="G", space="PSUM")
                nc.tensor.matmul(out=gz[:, :w], lhsT=wmb("WgT_z"),
                                 rhs=ssb[:, :w], start=True, stop=True)
                gn = qpool.tile([P, GW], f32, tag="G", space="PSUM")
                nc.tensor.matmul(out=gn[:, :w], lhsT=wmb("WgT_n"),
                                 rhs=ssb[:, :w], start=True, stop=False)

                rsb = spool.tile([P, GW], bf, tag="rsb")
                nc.scalar.activation(out=rsb[:, :w], in_=gr[:, :w],
                                     func=AF.Sigmoid, bias=vcc("bias_r"))
                zsb = spool.tile([P, GW], bf, tag="zsb")
                nc.scalar.activation(out=zsb[:, :w], in_=gz[:, :w],
                                     func=AF.Sigmoid, bias=vcc("nbias_z"),
                                     scale=-1.0)
                nc.tensor.matmul(out=gn[:, :w], lhsT=wmb("diag_hn"),
                                 rhs=rsb[:, :w], start=False, stop=True)
                nsb = spool.tile([P, GW], bf, tag="nsb")
                nc.scalar.activation(out=nsb[:, :w], in_=gn[:, :w],
                                     func=AF.Tanh, bias=vcc("bias_n"))

                t3 = spool.tile([P, GW], bf, tag="t3")
                veng.tensor_scalar(out=t3[:, :w], in0=nsb[:, :w],
                                   scalar1=vcc("h0"), scalar2=None,
                                   op0=OP.subtract)
                t4 = spool.tile([P, GW], bf, tag="t4")
                veng.tensor_tensor(out=t4[:, :w], in0=t3[:, :w],
                                   in1=zsb[:, :w], op=OP.mult)
                hn = hpool.tile([P, GW], bf, tag="hnew")
                veng.tensor_scalar(out=hn[:, :w], in0=t4[:, :w],
                                   scalar1=vcc("h0"), scalar2=None,
                                   op0=OP.add)
                hnew.append(hn)

                # stage this group for the AllGather as soon as its h_new
                # is ready (transpose to node-major, copy, DMA); only the
                # sourced prefix (slots < K[l]) is ever gathered later
                if l < NL - 1 and g * GW < int(K[l]):
                    agt = ag_in[l % 2]
                    tp = tpool.tile([P, GW], bf, tag="tp", space="PSUM")
                    nb = w // P
                    for b in range(nb):
                        if g * GW + b * P >= int(K[l]):
                            break
                        nc.tensor.transpose(
                            out=tp[:, b * P:(b + 1) * P],
                            in_=hn[:, b * P:(b + 1) * P],
                            identity=wmb("ident"))
                    nb2 = min(nb, (int(K[l]) - g * GW) // P)
                    tps = spool.tile([P, GW], bf, tag="tps")
                    nc.vector.tensor_copy(out=tps[:, :nb2 * P],
                                          in_=tp[:, :nb2 * P])
                    for b in range(nb2):
                        row = g * GW + b * P
                        nc.sync.dma_start(out=agt[row:row + P, :],
                                          in_=tps[:, b * P:(b + 1) * P])

            if l < NL - 1 and int(K[l]) > 0:
                cc = nc.gpsimd.collective_compute(
                    "AllGather", mybir.AluOpType.bypass,
                    replica_groups=RG,
                    ins=[agt[0:int(K[l]), :].opt()],
                    outs=[h_store[int(CL_off[l]):int(CL_off[l])
                                  + NC * int(K[l]), :].opt()],
                )
                last_ag[0] = cc

            # MLP head for this level (fills the AllGather latency)
            for g, w in enumerate(widths):
                emit_mlp(l, g, w, hnew[g])

            # seeds + old chunks for the next level (also fill the AllGather)
            if l + 1 < NL:
                emit_seeds(l + 1)
                oh_o, p0_o = OH.get((l + 1, "o"), (None, None))
                if Old_sb[l + 1] is not None:
                    emit_chunks(l + 1, "old", Old_sb[l + 1], oh_o, p0_o)

            prefetch_oh(l + 1)

        for pl in (rpool, tpool, qpool, ppool, hpool, gpool, spool, cpool):
            pl.release()

    nc.compile()
    return nc


# ---------------------------------------------------------------------------
# Entry point
# ---------------------------------------------------------------------------

def _run(inputs, trace=False, reps=1):
    from concourse.bass_utils import run_bass_kernel_spmd

    inputs = {k: np.asarray(v) for k, v in inputs.items()}
    bf16 = _bf16()
    fl = np.asarray(inputs["forward_level"])
    num_levels = int(fl.max()) + 1
    sched = _preprocess(fl, inputs["edge_index"], num_levels)
    wmat, vcols, vrow = _prep_weights(inputs)

    key = (sched["N"], sched["ICOLS"], sched["NPAIR"], sched["sumVc"], reps,
           tuple(int(x) for x in sched["K"]),
           tuple(int(x) for x in sched["Vc"]),
           tuple((len(i["instrs"]), len(i["pairs"]))
                 for i in sched["levels"]))
    if key not in _COMPILED:
        _COMPILED[key] = _build(sched, reps=reps)
    nc = _COMPILED[key]

    in_maps = []
    for c in range(NC):
        in_maps.append({
            "wmat": wmat, "vcols": vcols, "vrow": vrow,
            "n0row": sched["n0row"][c][None, :].astype(bf16),
            "degrow": sched["degrow"][c][None, :].astype(bf16),
            "idxs": sched["idxs"][c],
            "ranks": sched["ranks"][c].astype(bf16),
        })

    res = run_bass_kernel_spmd(nc, in_maps, core_ids=list(range(NC)),
                               trace=trace)

    NL = sched["NL"]
    L_off, Vc, Voff = sched["L_off"], sched["Vc"], sched["Voff"]
    node_of_rank = sched["node_of_rank"]
    out = np.zeros(sched["N"], np.float32)
    for c in range(NC):
        oc = res.results[c]["pred"]
        for l in range(NL):
            gr = int(L_off[l]) + c * int(Vc[l]) + np.arange(int(Vc[l]))
            nd = node_of_rank[gr]
            m = nd >= 0
            out[nd[m]] = oc[int(Voff[l]):int(Voff[l]) + int(Vc[l])][m]
    return out[:, None], res


def kernel(**inputs):
    out, _ = _run(inputs, trace=False)
    return out
